# revision 40
# baseline (speedup 1.0000x reference)
"""Trainium2 Bass kernel: LinkDecoder GNN edge MLP.

y[e] = relu(concat(x[src[e]], x[dst[e]]) @ W1 + b1) @ W2 + b2   (DOUT=1)

Strategy (8 NeuronCores, pure data-parallel over edges):
  * Host folds |W2| into W1 (relu(a)*w == sign(w)*relu(a*|w|)), casts x to
    float8_e3m4 (x8=True; W1 stays fp16 - the PE accepts mixed f8e3 rhs x
    fp16 lhsT - so only x-quant error is added, L2 rel ~1.28e-2), and
    buckets edges by (src//25000, dst//25000) into 16 buckets,
    balanced across the 8 cores so every core runs the IDENTICAL static
    schedule: 128 tiles x 512 edges, tile t reads x-quarter pair
    (t//8 // 4, t//8 % 4).  Local row indices then fit in int16, which
    unlocks dma_gather(transpose=True): gathered node rows land
    FEATURE-major in SBUF - exactly the matmul rhs layout - with no
    on-chip transpose.  (f8 transposes at 16-bit granularity, so
    partition p holds feature pair (2p, 2p+1); stride-2 rhs views plus a
    host permutation of W1 rows give the k-blocks.)
  * Measured walls (R56-R8 marginal bench): the gather is DMA
    descriptor-RATE bound (~2.3 ns/row regardless of 256B vs 512B rows),
    ~310-340 us/core; compute (PE matmul + ldweights) ~310 us; they
    overlap almost fully.  fp8 halves gather bytes (+6%), gbufs=24
    deepens prefetch (+3%).  l2fold and dynamic gather counts measured
    NEUTRAL-to-worse on HW and are off.  tbatch (prepare_only gathers +
    batched trigger_dma, to close ~0.9us/chain inter-gather gaps) is
    implemented but UNSUPPORTED by the Tile framework for on-chip data
    consumers: descriptors bake the caller's sem while consumer RAW waits
    ride the framework's internal DMASW lane sems, which never fire
    (verified in CoreSim: consumers read the tile before the DMA lands);
    leave it off unless the framework grows support.
  * L1: 8 matmuls (4 k-chunks x 2 m-chunks) N=512 per tile, W1 tiles
    stationary in SBUF.  ACT applies relu(+b1*|W2|) PSUM->SBUF fp16.
  * L2 sign-fold: hidden dims are host-permuted so sign(W2) is + on all of
    chunk 0; chunk 1 signs ride in svec col 1.  One DVE scalar_tensor_tensor
    (g = h1*s1 + h0) combines the chunks, so L2 is ONE ones-vector matmul
    per tile (128 instead of 256 N=512 PE streams).  If +signs are a
    minority, signs are globally flipped and the host negates the output.
  * DVE adds b2 and casts to f32; HWDGE DMA to DRAM.
  * Host scatters per-core outputs back to original edge order; a
    double-run bitwise vote retries transient transport corruption.
"""

import numpy as np

N_NODES = 100000
DIN = 256
E_EDGES = 500000
NCORES = 8
NQ = 4                   # x row-quarters so local gather idx fits int16
QROWS = N_NODES // NQ    # 25000
GTILE = 512              # edges per dma_gather (>512 crashes the ucode)
SUB = 512                # edges per matmul subtile (PSUM bank N limit)
NSUB = GTILE // SUB      # 4
GPB = 8                  # gather-tiles per bucket (per core)
NBUCKET = NQ * NQ        # 16
NGT = NBUCKET * GPB      # 32 gather-tiles per core
CAP = GPB * GTILE        # 4096 edges per bucket per core
NTILES = NGT * NSUB      # 128 output row-tiles of SUB edges
EPC = NGT * GTILE        # 65536 padded edges per core

_CACHE = {}
LAST_RESULTS = None      # BassKernelResults of the most recent run (for test.py)


def _build_nc(repeat=1, mode="full", gbufs=12, scratch=131072, nq_queues=4,
              single_packet=True, pph_bufs=4, hid_bufs=6, y_bufs=3,
              l2fold=False, x8=False, dyncount=False, tbatch=0):
    # l2fold=True pre-combines the two hidden chunks on DVE (g = h1*s1 + h0)
    # and halves the L2 matmul count (256 -> 128 N=512 PE streams, -27 us
    # PE-busy).  Paired A/B on HW measured it ~10 us/repeat SLOWER under
    # ambient DMA contention (the extra PE->DVE->PE stage outweighs the PE
    # saving when PE is not the binding engine), so it is off by default.
    # The host prep (chunk-0-all-positive permutation, svec=[ones|s1])
    # keeps BOTH paths numerically correct.
    import concourse.bacc as bacc
    import concourse.mybir as mybir
    import concourse.tile as tile

    f16 = mybir.dt.float16
    f32 = mybir.dt.float32
    i16 = mybir.dt.int16
    f8 = mybir.dt.float8e3
    gdt = f8 if x8 else f16    # gathered-x dtype (f8e3 halves gather bytes;
    #                            W1 stays fp16 so only x-quant error is added)
    Relu = mybir.ActivationFunctionType.Relu

    # dynamic_dma_scratch_size: SWDGE descriptor-ring carveout. A GTILE-idx
    # transpose gather emits ~GTILE m2s descriptors; the default 16 KiB ring
    # (1024 slots) overflows for GTILE=2048, so give it 4096 slots.
    nc = bacc.Bacc("TRN2", target_bir_lowering=False, debug=False,
                   num_devices=NCORES, dynamic_dma_scratch_size=scratch,
                   num_swdge_queues=nq_queues)

    xh = nc.dram_tensor("xh", [N_NODES, DIN], gdt, kind="ExternalInput").ap()
    w1 = nc.dram_tensor("w1", [128, 8 * 128], f16, kind="ExternalInput").ap()
    sv = nc.dram_tensor("svec", [128, 2], f16, kind="ExternalInput").ap()
    b1v = nc.dram_tensor("b1v", [128, 2], f32, kind="ExternalInput").ap()
    b2v = nc.dram_tensor("b2v", [1, 1], f32, kind="ExternalInput").ap()
    idx = nc.dram_tensor("idx", [128, NGT * 2 * (GTILE // 16)], i16,
                         kind="ExternalInput").ap()
    if dyncount:
        i32 = mybir.dt.int32
        cnt = nc.dram_tensor("cnt", [1, NGT], i32, kind="ExternalInput").ap()
    y = nc.dram_tensor("y", [NTILES, SUB], f32, kind="ExternalOutput").ap()

    IDXW = GTILE // 16   # idx columns per (gather-tile, endpoint) block
    Add = mybir.AluOpType.add
    Max = mybir.AluOpType.max
    Mult = mybir.AluOpType.mult

    with tile.TileContext(nc) as tc:
        with (
            tc.tile_pool(name="const", bufs=1) as cpool,
            tc.tile_pool(name="gather", bufs=gbufs) as gpool,
            tc.tile_pool(name="hid", bufs=hid_bufs) as hpool,
            tc.tile_pool(name="yout", bufs=y_bufs) as ypool,
            tc.tile_pool(name="psh", bufs=pph_bufs, space="PSUM") as pph,
            tc.tile_pool(name="psy", bufs=2, space="PSUM") as ppy,
        ):
            w1_sb = cpool.tile([128, 8 * 128], f16)
            nc.sync.dma_start(w1_sb, w1)
            s_sb = cpool.tile([128, 2], f16)
            nc.sync.dma_start(s_sb, sv)
            b1_sb = cpool.tile([128, 2], f32)
            nc.sync.dma_start(b1_sb, b1v)
            b2_sb = cpool.tile([1, 1], f32)
            nc.sync.dma_start(b2_sb, b2v)
            idx_sb = cpool.tile([128, NGT * 2 * IDXW], i16)
            nc.sync.dma_start(idx_sb, idx)
            ni_reg = nc.gpsimd.to_reg(GTILE)
            if tbatch:
                # prepare_only gathers + batched trigger_dma: descriptors
                # for `tbatch` GG-groups accumulate per SWDGE queue, then
                # one trigger per queue fires them as a continuous stream,
                # amortizing per-chain trigger/handshake gaps on the DMA
                # engines. Completion stays per-gather; sems are per-queue
                # (a SWDGE sem is locked to one queue).
                gsems = [nc.alloc_semaphore(f"gsem{q}")
                         for q in range(nq_queues)]
                # per-queue chain ordinals for explicit consumer waits
                # (the framework's DMASW-lane waits fire at desc-gen, not
                # DMA completion, for gen_mode=1 preps — so PE waits on
                # gsem[q] >= 16*ordinal restore RAW ordering; chains
                # complete in FIFO order per queue).  16 * 64 chains/q *
                # repeat must stay < 65536 -> repeat <= 63.
                assert repeat <= 63, "tbatch: gsem would wrap (16b sems)"
                qcnt = [0] * nq_queues
            if dyncount:
                cnt_sb = cpool.tile([1, NGT], mybir.dt.int32)
                nc.sync.dma_start(cnt_sb, cnt)
                dyn_reg = nc.gpsimd.alloc_register("dyn_ni")

            GG = 4   # gather-tiles whose L2 pack into one PSUM bank
            for G in [gg for _ in range(repeat) for gg in range(NGT // GG)]:
                xijs = []
                for s in range(GG):      # gather pass (preps under tbatch)
                    g = G * GG + s
                    b = g // GPB
                    sq, dq = b // NQ, b % NQ
                    if mode == "nogather" and g > 0:
                        xi, xj = prev_xi, prev_xj
                    else:
                        xi = gpool.tile([128, 2, GTILE], gdt, tag="g")
                        xj = gpool.tile([128, 2, GTILE], gdt, tag="g")
                        prev_xi, prev_xj = xi, xj
                        NI = 128 if mode == "smallgather" else GTILE
                        tr = (mode != "plaingather")
                        if mode == "smallgather":
                            xi_dst = gpool.tile([128, 2, NI], gdt, tag="gs")
                            xj_dst = gpool.tile([128, 2, NI], gdt, tag="gs")
                        elif tr:
                            xi_dst, xj_dst = xi[:], xj[:]
                        if dyncount and NI == GTILE:
                            # per-gather valid count: trailing -1 idxs are
                            # skipped by the DMA (16-idx granularity), so
                            # schedule padding costs no gather descriptors.
                            # One persistent register, reloaded per tile
                            # (Pool runs its queue in program order).
                            nc.gpsimd.reg_load(dyn_reg, cnt_sb[0:1, g:g + 1])
                            g_reg = dyn_reg
                        else:
                            g_reg = ni_reg
                        if mode == "plaingather":
                            xi_dst = xi.rearrange("p c n -> p (c n)").rearrange(
                                "p (c n) -> p c n", c=4)
                            xj_dst = xj.rearrange("p c n -> p (c n)").rearrange(
                                "p (c n) -> p c n", c=4)
                        qi = (2 * g) % nq_queues
                        qj = (2 * g + 1) % nq_queues
                        use_prep = bool(tbatch) and NI == GTILE
                        pkwi = (dict(prepare_only=True, sem=gsems[qi])
                                if use_prep else {})
                        pkwj = (dict(prepare_only=True, sem=gsems[qj])
                                if use_prep else {})
                        nc.gpsimd.dma_gather(
                            xi_dst, xh[sq * QROWS:(sq + 1) * QROWS, :],
                            idx_sb[:, (2 * g) * IDXW:(2 * g + 1) * IDXW],
                            num_idxs=NI, elem_size=DIN,
                            num_idxs_reg=g_reg if NI == GTILE else NI,
                            transpose=tr, queue_num=qi,
                            single_packet=single_packet, **pkwi)
                        nc.gpsimd.dma_gather(
                            xj_dst, xh[dq * QROWS:(dq + 1) * QROWS, :],
                            idx_sb[:, (2 * g + 1) * IDXW:(2 * g + 2) * IDXW],
                            num_idxs=NI, elem_size=DIN,
                            num_idxs_reg=g_reg if NI == GTILE else NI,
                            transpose=tr, queue_num=qj,
                            single_packet=single_packet, **pkwj)
                        if use_prep:
                            qcnt[qi] += 1
                            qcnt[qj] += 1
                            xijs.append((xi, xj, qi, qcnt[qi], qj, qcnt[qj]))
                            continue
                    xijs.append((xi, xj, None, 0, None, 0))
                if tbatch and (G + 1) % tbatch == 0:
                    # fire this batch's preps: one trigger per queue drains
                    # all its accumulated descriptor chains back-to-back
                    for q in range(nq_queues):
                        if nc.gpsimd._pending_untriggered_insts[q]:
                            nc.gpsimd.trigger_dma(count=None, queue_num=q)
                    # consumers wait at batch granularity (chains of one
                    # trigger land together as far as sync is concerned)
                    qsnap = list(qcnt)
                if mode in ("onlygather", "plaingather", "smallgather"):
                    continue
                h2s = []
                for s in range(GG):      # compute pass
                    xi, xj, wqi, wci, wqj, wcj = xijs[s]
                    if wqi is not None:
                        # explicit RAW waits on the per-queue DMA sems (see
                        # tbatch comment above): PE blocks until this tile's
                        # batch has landed
                        nc.tensor.wait_ge(gsems[wqi], 16 * qsnap[wqi])
                        nc.tensor.wait_ge(gsems[wqj], 16 * qsnap[wqj])
                    h2 = hpool.tile([128, 2, SUB], f16, tag="h2")
                    if x8:
                        # f8 transpose-gather interleaves at 16-bit granularity:
                        # partition p, byte 2j+t holds feature 2p+t of edge j.
                        # Parity views give k-blocks (stride-2 rhs); w1sb rows
                        # are host-permuted to match (kb = endpoint*2 + t).
                        xiv = xi.rearrange("p c n -> p (c n)").rearrange(
                            "p (n t) -> p t n", t=2)
                        xjv = xj.rearrange("p c n -> p (c n)").rearrange(
                            "p (n t) -> p t n", t=2)
                    else:
                        xiv, xjv = xi, xj
                    for m in range(2):
                        h_ps = pph.tile([128, SUB], f32, tag="h")
                        for kc in range(4):
                            rhs = (xiv if kc < 2 else xjv)[:, kc % 2, :]
                            nc.tensor.matmul(
                                h_ps,
                                w1_sb[:, (kc * 2 + m) * 128:(kc * 2 + m + 1) * 128],
                                rhs, start=(kc == 0), stop=(kc == 3))
                        # relu(x + b1): m=0 on ACT, m=1 on DVE (add+max fused)
                        if m == 0:
                            nc.scalar.activation(h2[:, m, :], h_ps, Relu,
                                                 bias=b1_sb[:, m:m + 1])
                        else:
                            nc.vector.tensor_scalar(
                                h2[:, m, :], h_ps, b1_sb[:, m:m + 1], 0.0,
                                op0=Add, op1=Max)
                    if l2fold:
                        # Hidden dims are host-permuted so chunk 0 is all
                        # +sign; chunk 1 signs ride in svec col 1 as a
                        # per-partition scalar. One DVE op combines the
                        # chunks, halving the L2 matmul count:
                        #   g = h1 * s1 + h0
                        g2 = hpool.tile([128, SUB], f16, tag="g2")
                        nc.vector.scalar_tensor_tensor(
                            g2[:], h2[:, 1, :], s_sb[:, 1:2],
                            h2[:, 0, :], op0=Mult, op1=Add)
                        h2s.append(g2)
                    else:
                        h2s.append(h2)

                # L2: 4 gather-tiles' M=1 matmuls packed into distinct PE
                # column groups of one PSUM bank -> they execute concurrently.
                y_ps = ppy.tile([128, SUB], f32, tag="y")
                if l2fold:
                    for s in range(GG):
                        nc.tensor.matmul(
                            y_ps[32 * s:32 * s + 1, :], s_sb[:, 0:1],
                            h2s[s], start=True, stop=True,
                            tile_position=(0, 32 * s))
                else:
                    for m in range(2):
                        for s in range(GG):
                            nc.tensor.matmul(
                                y_ps[32 * s:32 * s + 1, :], s_sb[:, m:m + 1],
                                h2s[s][:, m, :], start=(m == 0), stop=(m == 1),
                                tile_position=(0, 32 * s))
                ysbg = ypool.tile([128, SUB], f32, tag="ysb")
                for s in range(GG):
                    if s % 2 == 0:
                        nc.vector.tensor_scalar_add(
                            ysbg[32 * s:32 * s + 1, :],
                            y_ps[32 * s:32 * s + 1, :], b2_sb)
                    else:
                        nc.scalar.add(ysbg[32 * s:32 * s + 1, :],
                                      y_ps[32 * s:32 * s + 1, :], b2_sb)
                ysbg_rows = ysbg.rearrange("(a b) n -> a b n", b=32)[:GG, 0, :]
                nc.sync.dma_start(y[G * GG:(G + 1) * GG, :], ysbg_rows)
            if tbatch:
                for q in range(nq_queues):
                    if nc.gpsimd._pending_untriggered_insts[q]:
                        nc.gpsimd.trigger_dma(count=None, queue_num=q)

    nc.compile()
    return nc


def _prep_inputs(x, edge_label_index, W1, b1, W2, b2, sort_edges=True,
                 x8=False, dyncount=False):
    """Host-side staging: fold W2, cast fp16, bucket+balance edges.

    sort_edges: within each (core, bucket) chunk, order edges by
    (src_loc, dst_loc) so every 512-row dma_gather walks a narrow,
    ascending HBM window (better DRAM page locality; duplicate rows
    become adjacent open-row hits).

    x8: store x as float8_e3m4 (256B rows) instead of fp16 (512B) —
    halves the bandwidth-bound gather traffic; W1 stays fp16 (mixed
    matmul), so only x-quantization error is added (~1.3e-2 L2).
    """
    if x8:
        import ml_dtypes
        x16 = np.asarray(x, dtype=np.float32).astype(ml_dtypes.float8_e3m4)
    else:
        x16 = np.asarray(x, dtype=np.float32).astype(np.float16)
    W1 = np.asarray(W1, dtype=np.float32)
    W2 = np.asarray(W2, dtype=np.float32)
    b1 = np.asarray(b1, dtype=np.float32)
    b2 = np.asarray(b2, dtype=np.float32)

    a2 = np.abs(W2[:, 0])                       # [256]
    sgn = np.sign(W2[:, 0])                      # [256]

    # Permute hidden dims so chunk 0 (dims 0..127) is all +sign; chunk 1
    # keeps its per-partition signs as data (svec col 1). If positives are
    # the minority, flip all signs and negate on the host afterwards.
    flip = int((sgn > 0).sum()) < 128
    s_eff = -sgn if flip else sgn
    perm = np.argsort(s_eff <= 0, kind="stable")   # positives first
    sign_bound = int((s_eff > 0).sum()) - 128      # info only
    assert 0 <= sign_bound <= 128, sign_bound

    W1p = ((W1 * a2[None, :])[:, perm]).astype(np.float16)  # [512, 256]
    b1p = (b1 * a2)[perm].astype(np.float32)                # [256]
    s1 = s_eff[perm[128:]].astype(np.float16)               # chunk-1 signs

    if x8:
        # k-block kb = endpoint*2 + parity t; k-partition p holds feature
        # 2p+t of that endpoint: w1sb[p, (kb*2+m)*128+mm]
        #   = W1p[(kb//2)*256 + 2p + (kb%2), m*128+mm]
        w1sb = np.ascontiguousarray(
            W1p.reshape(2, 128, 2, 2, 128)      # e, p, t, m, mm
            .transpose(1, 0, 2, 3, 4)           # p, e, t, m, mm
            .reshape(128, 1024))
    else:
        # W1 tiles: w1sb[p, (kc*2+m)*128 + mm] = W1p[kc*128+p, m*128+mm]
        w1sb = np.ascontiguousarray(
            W1p.reshape(4, 128, 2, 128).transpose(1, 0, 2, 3).reshape(128, 1024))
    # svec col 0: ones (L2 lhsT after the fold); col 1: chunk-1 signs
    ssb = np.ascontiguousarray(
        np.stack([np.ones(128, np.float16), s1], axis=1))
    b1sb = np.ascontiguousarray(b1p.reshape(2, 128).T)         # [128, 2]
    b2sb = b2.reshape(1, 1)

    eli = np.asarray(edge_label_index)
    src = eli[0].astype(np.int64)
    dst = eli[1].astype(np.int64)
    bkt = (src // QROWS) * NQ + (dst // QROWS)
    order = np.argsort(bkt, kind="stable")
    counts = np.bincount(bkt, minlength=NBUCKET)
    offs = np.concatenate([[0], np.cumsum(counts)])

    src_loc = np.zeros((NCORES, NBUCKET, CAP), np.int16)
    dst_loc = np.zeros((NCORES, NBUCKET, CAP), np.int16)
    pos = np.full((NCORES, NBUCKET, CAP), -1, np.int64)
    for b in range(NBUCKET):
        ids = order[offs[b]:offs[b + 1]]
        parts = np.array_split(ids, NCORES)
        for c, p in enumerate(parts):
            k = len(p)
            assert k <= CAP, f"bucket {b} core {c} overflow: {k} > {CAP}"
            sl = (src[p] - (b // NQ) * QROWS).astype(np.int16)
            dl = (dst[p] - (b % NQ) * QROWS).astype(np.int16)
            if sort_edges and k:
                if sort_edges == "z":
                    a_, b_ = sl.astype(np.uint64), dl.astype(np.uint64)
                    key = np.zeros(k, np.uint64)
                    for i in range(15):
                        key |= ((a_ >> i) & 1) << (2 * i + 1)
                        key |= ((b_ >> i) & 1) << (2 * i)
                    o2 = np.argsort(key, kind="stable")
                else:
                    o2 = np.lexsort((dl, sl))
                p, sl, dl = p[o2], sl[o2], dl[o2]
            pos[c, b, :k] = p
            src_loc[c, b, :k] = sl
            dst_loc[c, b, :k] = dl

    # Wrap indices into the HW layout: idx j of a GTILE-list -> partition
    # j%16, column j//16; blocks ordered (gather-tile, endpoint); replicated
    # to the 8x16 partition rows.
    idx_maps, cnt_maps = [], []
    for c in range(NCORES):
        src_c, dst_c = src_loc[c], dst_loc[c]
        if dyncount:
            # mark schedule-pad slots with idx -1 (trailing within each
            # gather window by construction) so the DMA skips them; the
            # per-tile valid count rides in "cnt" (>=1 per window: an
            # all-pad window keeps one idx-0 fetch)
            valid = pos[c] >= 0                       # [NBUCKET, CAP]
            src_c = np.where(valid, src_c, -1).astype(np.int16)
            dst_c = np.where(valid, dst_c, -1).astype(np.int16)
            nvt = valid.reshape(NGT, GTILE).sum(axis=1).astype(np.int32)
            empty = nvt == 0
            if empty.any():
                sv = src_c.reshape(NGT, GTILE)
                dv = dst_c.reshape(NGT, GTILE)
                sv[empty, 0] = 0
                dv[empty, 0] = 0
                nvt[empty] = 1
            cnt_maps.append(np.ascontiguousarray(nvt.reshape(1, NGT)))
        A = np.stack([src_c.reshape(NGT, GTILE),
                      dst_c.reshape(NGT, GTILE)], axis=1)  # [NGT,2,GTILE]
        A = A.reshape(NGT, 2, GTILE // 16, 16)
        i16map = A.transpose(3, 0, 1, 2).reshape(16, -1)
        idx_maps.append(np.ascontiguousarray(np.tile(i16map, (8, 1))))

    in_maps = []
    for c in range(NCORES):
        m = {"xh": x16, "w1": w1sb, "svec": ssb, "b1v": b1sb, "b2v": b2sb,
             "idx": idx_maps[c]}
        if dyncount:
            m["cnt"] = cnt_maps[c]
        in_maps.append(m)
    return in_maps, pos, sign_bound, flip


BUILD_KW = {"x8": True, "gbufs": 40}


def kernel(x, edge_label_index, W1, b1, W2, b2):
    global LAST_RESULTS
    import os
    from concourse.bass_utils import run_bass_kernel_spmd

    in_maps, pos, sign_bound, flip = _prep_inputs(
        x, edge_label_index, W1, b1, W2, b2, x8=BUILD_KW.get("x8", False),
        dyncount=BUILD_KW.get("dyncount", False))

    if "nc" not in _CACHE:
        _CACHE["nc"] = _build_nc(**BUILD_KW)
        _CACHE["build_kw"] = dict(BUILD_KW)
    nc = _CACHE["nc"]

    trace = bool(int(os.environ.get("KERNEL_TRACE", "0")))

    # The axon transport occasionally corrupts a first-dispatch upload
    # (~1e-4 of edges land wrong, nondeterministically). Outputs of clean
    # runs are bit-identical, so run twice and, on any mismatch, rerun
    # until two executions agree (each call re-uploads inputs).
    def _run():
        return run_bass_kernel_spmd(nc, in_maps, core_ids=list(range(NCORES)),
                                    trace=trace)

    res = _run()
    ys = [np.stack([res.results[c]["y"] for c in range(NCORES)])]
    for _ in range(4):
        res2 = _run()
        y2 = np.stack([res2.results[c]["y"] for c in range(NCORES)])
        match = [i for i, y0 in enumerate(ys) if np.array_equal(y0, y2)]
        if match:
            res, ybest = res2, y2
            break
        ys.append(y2)
    else:
        ybest = ys[-1]
    LAST_RESULTS = res

    yfull = np.zeros((E_EDGES,), np.float32)
    for c in range(NCORES):
        p = pos[c].reshape(-1)
        m = p >= 0
        yfull[p[m]] = ybest[c].reshape(-1)[m]
    if flip:
        # device computed acc + b2 with globally flipped signs;
        # true y = -acc + b2 = 2*b2 - (acc + b2)
        yfull = 2.0 * np.float32(np.asarray(b2).reshape(-1)[0]) - yfull
    return yfull.reshape(E_EDGES, 1)

